# revision 5
# baseline (speedup 1.0000x reference)
"""Masked attention kernel for Trainium2, SPMD over 8 NeuronCores.

Problem: B=4, H=16, S=2048, D=64 attention with a [B,1,S,S] bool mask
(True = masked out).  The 64 (b,h) pairs are fully independent; core c
handles pairs c*8..c*8+7, which all share batch b=c//2, so each core
loads exactly one batch's mask.

Device-side math per (b,h), all in "transposed" layout (no on-device
transposes; the host pre-transposes Q/K/mask and post-transposes out):

    ST[k, q] = K @ Q^T            (f16 matmul, f32 PSUM accumulate)
    ET[k, q] = exp(ST / 8)        (ScalarE, written as f16)
    PT[k, q] = ET * keepT[k, q]   (VectorE; keepT = !mask as f16 -> exact
                                   zeros for masked entries)
    OT[d, q] = V1^T @ PT          (V1 = [V | ones] -> row 64 of OT is the
                                   softmax denominator sum_k PT[k, q])

Host then returns (OT[:64] / OT[64]).T per pair.  Skipping the softmax
max-subtraction is safe: scores/8 ~ N(0,1) so exp() cannot overflow, and
masked entries are exactly zero via the keep-mask multiply.

This runtime executes every instruction at a roughly size-independent
~40-90us (engines do not overlap; cores do run in parallel), so the
kernel minimizes INSTRUCTION COUNT per core:
  - 512 score matmuls (64/pair: 16 k-chunks x 4 q-tiles of 512)  [floor]
  - 512 PV matmuls    (64/pair: 16 k-chunks x 4 q-tiles)         [floor]
  - 64 exps (8/pair: one [128,4096] activation per 2 chunks, reading
    the full 8-bank PSUM in one instruction)
  - 8 mask multiplies (1/pair over the whole [128, 16, 2048] pt)
  - PV accumulates into a [65,2048] view of the SAME psum tile the
    scores used (engines are serial, so no pipelining is lost)
  - one batched output DMA per iteration ([65, 8, 2048] f32)
"""

import numpy as np

B, H, S, D = 4, 16, 2048, 64
NCORES = 8
PAIRS_PER_CORE = (B * H) // NCORES  # 8
QTW = 512    # matmul moving-operand width (psum bank cap)

F16 = np.float16

_CACHE = {}


def build_nc_tile(npairs=PAIRS_PER_CORE, s=S, niters=1):
    """Instruction-count-minimized Tile build (see module docstring)."""
    import concourse.bass as bass
    import concourse.bacc as bacc
    import concourse.tile as tile
    from concourse import mybir

    nchunk = s // 128           # 16 k-chunks of 128
    nqt = s // QTW              # 4 q-tiles of 512
    dt = mybir.dt

    nc = bacc.Bacc("TRN2", target_bir_lowering=False, debug=False,
                   num_devices=NCORES)

    qk_d = nc.dram_tensor("qk", [npairs, 64, 2 * s], dt.float16,
                          kind="ExternalInput")
    v1_d = nc.dram_tensor("v1", [npairs, 128, nchunk * 65], dt.float16,
                          kind="ExternalInput")
    mk_d = nc.dram_tensor("mk", [128, nchunk, s], dt.float16,
                          kind="ExternalInput")
    ot_d = nc.dram_tensor("ot", [65, npairs, s], dt.float32,
                          kind="ExternalOutput")

    with tile.TileContext(nc) as tc:
        with (
            tc.tile_pool(name="const", bufs=1) as const_pool,
            tc.tile_pool(name="qk", bufs=1) as qk_pool,
            tc.tile_pool(name="v", bufs=1) as v_pool,
            tc.tile_pool(name="p", bufs=1) as p_pool,
            tc.tile_pool(name="osb", bufs=1) as o_pool,
            tc.tile_pool(name="sc", bufs=1, space=bass.MemorySpace.PSUM) as sc_pool,
        ):
            mk_t = const_pool.tile([128, nchunk, s], dt.float16)
            nc.sync.dma_start(mk_t[:], mk_d[:])
            osb = o_pool.tile([65, npairs, s], dt.float32)

            for it in range(niters):
                for p in range(npairs):
                    qk_t = qk_pool.tile([64, 2 * s], dt.float16)
                    nc.sync.dma_start(qk_t[:], qk_d[p])
                    v1_t = v_pool.tile([128, nchunk, 65], dt.float16)
                    nc.sync.dma_start(
                        v1_t[:].rearrange("p c m -> p (c m)"),
                        v1_d[p])

                    pt = p_pool.tile([128, nchunk, s], dt.float16)
                    sc = sc_pool.tile([128, 2 * s], dt.float32)  # 8 banks

                    # scores + exp, two k-chunks per activation
                    for c0 in range(0, nchunk, 2):
                        for ci in range(2):
                            c = c0 + ci
                            for t in range(nqt):
                                nc.tensor.matmul(
                                    sc[:, ci * s + t * QTW:
                                       ci * s + (t + 1) * QTW],
                                    qk_t[:, s + c * 128:s + (c + 1) * 128],
                                    qk_t[:, t * QTW:(t + 1) * QTW],
                                    start=True, stop=True,
                                )
                        nc.scalar.activation(
                            pt[:, c0:c0 + 2, :].rearrange("p c q -> p (c q)"),
                            sc[:],
                            mybir.ActivationFunctionType.Exp, scale=0.125,
                        )

                    # mask multiply, all 16 chunks in one instruction
                    nc.vector.tensor_mul(pt[:], pt[:], mk_t[:])

                    # PV accumulates into a [65, s] view of the score psum
                    acc = sc[0:65, 0:s]
                    for c in range(nchunk):
                        for t in range(nqt):
                            nc.tensor.matmul(
                                acc[:, t * QTW:(t + 1) * QTW],
                                v1_t[:, c, :],
                                pt[:, c, t * QTW:(t + 1) * QTW],
                                start=(c == 0), stop=(c == nchunk - 1),
                            )
                    nc.vector.tensor_copy(osb[:, p, :], acc)

                # one batched output DMA per iteration
                nc.sync.dma_start(ot_d[:], osb[:])

    nc.compile()
    return nc


def build_current(niters=1):
    return build_nc_tile(PAIRS_PER_CORE, S, niters)


def _get_nc():
    key = (PAIRS_PER_CORE, S)
    if key not in _CACHE:
        _CACHE[key] = build_current()
    return _CACHE[key]


def _pack_v_chunks(v, ones_val, s):
    """[s, 64] -> [128, nchunk*65] chunk layout with a constant 65th column."""
    nchunk = s // 128
    vc = v.reshape(nchunk, 128, 64).transpose(1, 0, 2)
    col = np.full((128, nchunk, 1), ones_val, dtype=v.dtype)
    return np.concatenate([vc, col], axis=2).reshape(128, nchunk * 65)


def make_core_inputs(Q, K, V, mask, core, npairs=PAIRS_PER_CORE, s=S, fmt=None):
    """Host-side shard prep for one core (numpy only)."""
    nchunk = s // 128
    pairs = [(f // H, f % H) for f in range(core * npairs, (core + 1) * npairs)]
    b0 = pairs[0][0]

    qk = np.empty((npairs, 64, 2 * s), dtype=F16)
    v1 = np.empty((npairs, 128, nchunk * 65), dtype=F16)
    for i, (b, h) in enumerate(pairs):
        qk[i, :, 0:s] = Q[b, h].T.astype(F16)
        qk[i, :, s:] = K[b, h].T.astype(F16)
        v1[i] = _pack_v_chunks(V[b, h].astype(F16), 1.0, s)

    keep = (~mask[b0, 0].T).astype(F16)  # [k, q] 1.0 = keep, 0.0 = masked
    mk = np.ascontiguousarray(
        keep.reshape(nchunk, 128, s).transpose(1, 0, 2))  # [128, nchunk, s]
    return {"qk": qk, "v1": v1, "mk": mk}


def kernel(Q, K, V, mask):
    from concourse.bass_utils import run_bass_kernel_spmd

    Q = np.asarray(Q, dtype=np.float32)
    K = np.asarray(K, dtype=np.float32)
    V = np.asarray(V, dtype=np.float32)
    mask = np.asarray(mask)

    nc = _get_nc()
    in_maps = [make_core_inputs(Q, K, V, mask, c) for c in range(NCORES)]
    res = run_bass_kernel_spmd(nc, in_maps, list(range(NCORES)))

    out = np.empty((B, H, S, D), dtype=np.float32)
    for c in range(NCORES):
        ot = res.results[c]["ot"]  # [65, npairs, S]
        for i in range(PAIRS_PER_CORE):
            f = c * PAIRS_PER_CORE + i
            b, h = f // H, f % H
            denom = ot[64:65, i, :]
            denom = np.where(denom == 0.0, 1.0, denom)
            out[b, h] = (ot[:64, i, :] / denom).T
    return out
